# revision 4
# baseline (speedup 1.0000x reference)
"""Single-head attention (B=4, S=4096, E=1024, H=64) on 8 TRN2 NeuronCores.

Sharding: core c -> (batch b = c//2, sequence half h = c%2). No collectives:
each core receives the transposed bf16 x for its WHOLE batch row, laid out
own-half-first, computes K/V for the full 4096-key sequence plus Q for its
own 2048 queries, then runs attention and the output projection for its
queries. K/V duplication across the pair costs ~16k PE cycles -- far less
than the AllGather machinery it replaces. Softmax over keys is permutation
invariant, so the own-first key order needs no unpermute.

Matmuls are bf16 (fp8 was measured numerically dead for this problem: the
softmax is extremely peaked, Neff ~ 6, so e4m3 noise doesn't average out).
All large matmuls use full 128x128 stationary tiles (zero/junk padded) to
keep the PE at 2.4 GHz -- masked sub-tiles clock-gate the PE to 1.2 GHz.

The scalar (ACT) engine does nothing but the 64 softmax Exp instructions --
its 1 col/cycle @ 1.2 GHz on the 8.4M-element score matrix is the kernel's
floor -- everything else is pushed to DVE/Pool/Sync. Host-side layout work
(transpose + bf16 cast of x, weight packing, fp32 upcast of the bf16
output) keeps on-chip data movement minimal.

Output projection uses the augmented-row trick: W_out carries b_out as row
64 and the bf16 context carries the softmax denominator in row 64, so
(ctx_aug.T @ W_out_aug) * recip(denom) applies scale and bias in one pass
(denom * recip == 1)."""

import sys

import numpy as np

for _p in ("/opt/trn_rl_repo",):
    if _p not in sys.path:
        sys.path.insert(0, _p)

from contextlib import ExitStack

import ml_dtypes

import concourse.bass as bass  # noqa: F401  (import keeps bass registered)
import concourse.mybir as mybir
import concourse.tile as tile
from concourse import bacc, masks
from concourse.bass_utils import run_bass_kernel_spmd

F32 = mybir.dt.float32
BF16 = mybir.dt.bfloat16
AF = mybir.ActivationFunctionType

B, S, E, H = 4, 4096, 1024, 64
SH = S // 2           # queries per core
N_CORES = 8
ET = E // 128         # 8 embedding tiles
FC = 512              # projection chunk (cols of the seq axis)
NCH = S // FC         # 8 chunks over the full sequence
ST = S // 128         # 32 kj tiles over the full sequence
QC = 1024             # query chunk (one PSUM ctx tile)
SCALE = 0.125         # 1/sqrt(H)


def _emit(nc, tc, xt, wkv, wq, bkv, bq, wo, out_ext):
    with ExitStack() as top:
        const = top.enter_context(tc.tile_pool(name="const", bufs=1))

        ident = const.tile([128, 128], BF16)
        masks.make_identity(nc, ident[:])

        # Weights first on the sync HWDGE queue so the first projection
        # chunk isn't blocked behind 8 MiB of x.
        wkv_sb = const.tile([128, ET * 128], BF16)
        nc.sync.dma_start(wkv_sb[:], wkv[:, :])
        wq_sb = const.tile([128, ET * 128], BF16)
        nc.sync.dma_start(wq_sb[:], wq[:, :])
        bkv_sb = const.tile([128, 1], F32)
        nc.gpsimd.dma_start(bkv_sb[:], bkv.unsqueeze(1))
        bq_sb = const.tile([64, 1], F32)
        nc.gpsimd.dma_start(bq_sb[:], bq.unsqueeze(1))

        # Persistent operands. x_sb holds the 8 e-tiles side by side.
        x_sb = const.tile([128, ET * S], BF16)
        k2 = const.tile([128, S], BF16)     # kT on 0:64, zeros on 64:128
        q2 = const.tile([128, SH], BF16)    # qT on 0:64, zeros on 64:128
        vt_sb = const.tile([128, S], BF16)  # vT on rows 64:128 (PSUM-aligned)
        v_aug = const.tile([128, ST * 128], BF16)
        wo_sb = const.tile([128, E], BF16)
        ones11 = const.tile([1, 1], BF16)

        nc.gpsimd.memset(k2[64:128, :], 0.0)
        nc.gpsimd.memset(q2[64:128, :], 0.0)
        nc.gpsimd.memset(v_aug[:], 0.0)
        # ones column (index 64) of every kj tile; junk cols 65:128 stay 0
        nc.gpsimd.memset(
            v_aug[:].rearrange("p (t c) -> p t c", c=128)[:, :, 64:65], 1.0
        )
        nc.gpsimd.memset(ones11[:], 1.0)

        # x in 1024-col blocks x 8 e-slices so projection chunks can start
        # as soon as their columns land.
        for blk in range(S // 1024):
            f0 = blk * 1024
            for e in range(ET):
                nc.sync.dma_start(
                    x_sb[:, e * S + f0 : e * S + f0 + 1024],
                    xt[e * 128 : (e + 1) * 128, f0 : f0 + 1024],
                )
        nc.sync.dma_start(wo_sb[:], wo[:, :])  # needed only in phase C

        # ---- Phase A: QKV projection + V transpose ----------------------
        with ExitStack() as pa:
            mkvp = pa.enter_context(tc.tile_pool(name="mkv", bufs=2, space="PSUM"))
            mqp = pa.enter_context(tc.tile_pool(name="mq", bufs=2, space="PSUM"))
            vtp = pa.enter_context(tc.tile_pool(name="vtp", bufs=2, space="PSUM"))
            for c in range(NCH):
                f0 = c * FC
                mkv = mkvp.tile([128, FC], F32)
                for e in range(ET):
                    nc.tensor.matmul(
                        mkv[:],
                        wkv_sb[:, e * 128 : (e + 1) * 128],
                        x_sb[:, e * S + f0 : e * S + f0 + FC],
                        start=(e == 0), stop=(e == ET - 1),
                    )
                if c < 4:  # own half: also project Q
                    mq = mqp.tile([128, FC], F32)
                    for e in range(ET):
                        nc.tensor.matmul(
                            mq[:],
                            wq_sb[:, e * 128 : (e + 1) * 128],
                            x_sb[:, e * S + f0 : e * S + f0 + FC],
                            start=(e == 0), stop=(e == ET - 1),
                        )
                    nc.vector.tensor_scalar_add(
                        q2[0:64, f0 : f0 + FC], mq[0:64, :], bq_sb[:]
                    )
                nc.vector.tensor_scalar_add(
                    k2[0:64, f0 : f0 + FC], mkv[0:64, :], bkv_sb[0:64, :]
                )
                nc.vector.tensor_scalar_add(
                    vt_sb[64:128, f0 : f0 + FC], mkv[64:128, :], bkv_sb[64:128, :]
                )
                for t in range(FC // 128):
                    kj = c * (FC // 128) + t
                    vp = vtp.tile([128, 64], F32)
                    nc.tensor.matmul(
                        vp[:],
                        vt_sb[64:128, kj * 128 : (kj + 1) * 128],
                        ident[64:128, 64:128],
                    )
                    nc.vector.tensor_copy(v_aug[:, kj * 128 : kj * 128 + 64], vp[:])

        # ---- Phase B: scores -> exp -> ctx accumulation -----------------
        with ExitStack() as pbc:
            cps = pbc.enter_context(tc.tile_pool(name="cps", bufs=1, space="PSUM"))
            expp = pbc.enter_context(tc.tile_pool(name="expp", bufs=6))
            ctxs = [cps.tile([128, QC], F32, name=f"ctx{i}") for i in range(2)]

            with ExitStack() as pb:
                sps = pb.enter_context(tc.tile_pool(name="sps", bufs=2, space="PSUM"))
                for kj in range(ST):
                    lhs_k = k2[:, kj * 128 : (kj + 1) * 128]
                    lhs_v = v_aug[:, kj * 128 : (kj + 1) * 128]
                    for qix in range(2):
                        ctx = ctxs[qix]
                        q0 = qix * QC
                        sp = sps.tile([128, QC], F32)
                        for n in range(QC // 512):
                            nc.tensor.matmul(
                                sp[:, n * 512 : (n + 1) * 512],
                                lhs_k,
                                q2[:, q0 + n * 512 : q0 + (n + 1) * 512],
                            )
                        ex = expp.tile([128, QC], BF16)
                        nc.scalar.activation(ex[:], sp[:], AF.Exp, scale=SCALE)
                        for n in range(QC // 512):
                            nc.tensor.matmul(
                                ctx[:, n * 512 : (n + 1) * 512],
                                lhs_v,
                                ex[:, n * 512 : (n + 1) * 512],
                                start=(kj == 0), stop=(kj == ST - 1),
                                skip_group_check=True,
                            )

            # ---- Phase C: output projection -----------------------------
            with ExitStack() as pc:
                ops = pc.enter_context(tc.tile_pool(name="ops", bufs=2, space="PSUM"))
                rsps = pc.enter_context(tc.tile_pool(name="rsps", bufs=1, space="PSUM"))
                ctxp = pc.enter_context(tc.tile_pool(name="ctxp", bufs=2))
                rsp = pc.enter_context(tc.tile_pool(name="rsp", bufs=2))
                outp = pc.enter_context(tc.tile_pool(name="outp", bufs=3))
                for qix in range(2):
                    ctx = ctxs[qix]
                    q0 = qix * QC
                    # rows 65:128 are exact zeros (v_aug junk cols are 0)
                    ctx16 = ctxp.tile([128, QC], BF16, tag="ctx16")
                    nc.vector.tensor_copy(ctx16[:], ctx[:])
                    # transpose the bf16 denominator row via tiny PE matmuls;
                    # recip(bf16 denom) keeps denom*recip == 1 for the bias row
                    rs_row = rsp.tile([1, QC], BF16, tag="rsrow")
                    nc.sync.dma_start(rs_row[:], ctx16[64:65, :])
                    rs_ps = rsps.tile([128, QC // 128], F32, tag="rsps")
                    for cc in range(QC // 128):
                        nc.tensor.matmul(
                            rs_ps[:, cc : cc + 1],
                            rs_row[0:1, cc * 128 : (cc + 1) * 128],
                            ones11[:],
                        )
                    recip = rsp.tile([128, QC // 128], F32, tag="recip")
                    nc.vector.reciprocal(recip[:], rs_ps[:])

                    for cc in range(QC // 128):
                        out_sb = outp.tile([128, E], BF16)
                        for n in range(2):
                            op = ops.tile([128, 512], F32)
                            nc.tensor.matmul(
                                op[:],
                                ctx16[:, cc * 128 : (cc + 1) * 128],
                                wo_sb[:, n * 512 : (n + 1) * 512],
                            )
                            # Pool can't read PSUM; ACT is idle here, so
                            # alternate the recip-scale between DVE and ACT
                            if (cc + n) % 2 == 0:
                                nc.vector.tensor_scalar_mul(
                                    out_sb[:, n * 512 : (n + 1) * 512],
                                    op[:],
                                    recip[:, cc : cc + 1],
                                )
                            else:
                                nc.scalar.mul(
                                    out_sb[:, n * 512 : (n + 1) * 512],
                                    op[:],
                                    recip[:, cc : cc + 1],
                                )
                        nc.sync.dma_start(
                            out_ext[q0 + cc * 128 : q0 + (cc + 1) * 128, :],
                            out_sb[:],
                        )


_NC = None


def _get_nc():
    global _NC
    if _NC is None:
        nc = bacc.Bacc("TRN2", target_bir_lowering=False, debug=False,
                       num_devices=N_CORES)
        xt = nc.dram_tensor("xt", [E, S], BF16, kind="ExternalInput").ap()
        wkv = nc.dram_tensor("wkv", [128, ET * 128], BF16, kind="ExternalInput").ap()
        wq = nc.dram_tensor("wq", [128, ET * 128], BF16, kind="ExternalInput").ap()
        bkv = nc.dram_tensor("bkv", [128], F32, kind="ExternalInput").ap()
        bq = nc.dram_tensor("bq", [64], F32, kind="ExternalInput").ap()
        wo = nc.dram_tensor("wo", [128, E], BF16, kind="ExternalInput").ap()
        out_ext = nc.dram_tensor("out", [SH, E], BF16, kind="ExternalOutput").ap()
        with tile.TileContext(nc) as tc:
            _emit(nc, tc, xt, wkv, wq, bkv, bq, wo, out_ext)
        nc.compile()
        _NC = nc
    return _NC


last_results = None
last_tmpdir = None


def kernel(x, W_qkv, b_qkv, W_out, b_out):
    nc = _get_nc()
    bf = ml_dtypes.bfloat16
    x = np.asarray(x, dtype=np.float32)
    Wq = np.asarray(W_qkv, dtype=np.float32)
    b1 = np.asarray(b_qkv, dtype=np.float32)

    wkv = np.empty((128, ET * 128), dtype=bf)
    wq_p = np.zeros((128, ET * 128), dtype=bf)
    for e in range(ET):
        wkv[:, e * 128 : e * 128 + 64] = Wq[e * 128 : (e + 1) * 128, 64:128]
        wkv[:, e * 128 + 64 : (e + 1) * 128] = Wq[e * 128 : (e + 1) * 128, 128:192]
        wq_p[:, e * 128 : e * 128 + 64] = Wq[e * 128 : (e + 1) * 128, 0:64]
    bkv = np.concatenate([b1[64:128], b1[128:192]]).astype(np.float32)
    bq = np.ascontiguousarray(b1[0:64])
    wo = np.zeros((128, E), dtype=bf)
    wo[0:64] = np.asarray(W_out, dtype=np.float32)
    wo[64] = np.asarray(b_out, dtype=np.float32)

    shared = {"wkv": wkv, "wq": wq_p, "bkv": bkv, "bq": bq, "wo": wo}
    in_maps = []
    for c in range(N_CORES):
        b, h = divmod(c, 2)
        xb = x[b]
        xt = np.empty((E, S), dtype=bf)
        xt[:, 0:SH] = xb[h * SH : (h + 1) * SH].T
        xt[:, SH:S] = xb[(1 - h) * SH : (2 - h) * SH].T
        in_maps.append({"xt": xt, **shared})

    import os
    import tempfile
    import time

    tmpdir = os.environ.get("ATTN_TRACE_DIR") or tempfile.mkdtemp(prefix="attn_trace_")
    res = None
    for attempt in range(3):
        try:
            res = run_bass_kernel_spmd(
                nc, in_maps, core_ids=list(range(N_CORES)), tmpdir=tmpdir
            )
            break
        except Exception:
            # transient NRT_EXEC_UNIT_UNRECOVERABLE has been observed on a
            # first attempt; a clean retry recovers
            if attempt == 2:
                raise
            time.sleep(2.0)
    global last_results, last_tmpdir
    last_results = res
    last_tmpdir = tmpdir

    out = np.empty((B, S, E), dtype=np.float32)
    for c in range(N_CORES):
        b, h = divmod(c, 2)
        out[b, h * SH : (h + 1) * SH] = res.results[c]["out"].astype(np.float32)
    return out


# revision 5
# speedup vs baseline: 1.0042x; 1.0042x over previous
"""Single-head attention (B=4, S=4096, E=1024, H=64) on 8 TRN2 NeuronCores.

Sharding: core c -> (batch b = c//2, sequence half h = c%2). No collectives:
each core receives the transposed bf16 x for its WHOLE batch row, laid out
own-half-first, computes K/V for the full 4096-key sequence plus Q for its
own 2048 queries, then runs attention and the output projection for its
queries. Softmax over keys is permutation invariant, so the own-first key
order needs no unpermute.

Matmuls are bf16 (fp8 was measured numerically dead for this problem: the
softmax is extremely peaked, Neff ~ 6, so e4m3 noise doesn't average out).
All large matmuls use full 128x128 stationary tiles (zero/junk padded) to
keep the PE at speed -- masked sub-tiles clock-gate the PE.

Softmax exp is split across two engines: the ACT engine computes exact Exp
for ~2/3 of the score tiles, and the DVE computes the rest with a
bf16 Schraudolph approximation (i16 = 23.083*s + 16249; bitcast to bf16
is 2^(logit*log2e) with ~2.3% sawtooth error; measured end-to-end rel err
~8e-3, budget 2e-2). x input lands via two HWDGE queues (sync + scalar)
to halve the descriptor-generation serialization.

Output projection uses the augmented-row trick: W_out carries b_out as row
64 and the bf16 context carries the softmax denominator in row 64, so
(ctx_aug.T @ W_out_aug) * recip(denom) applies scale and bias in one pass
(denom * recip == 1)."""

import sys

import numpy as np

for _p in ("/opt/trn_rl_repo",):
    if _p not in sys.path:
        sys.path.insert(0, _p)

from contextlib import ExitStack

import ml_dtypes

import concourse.bass as bass  # noqa: F401  (import keeps bass registered)
import concourse.mybir as mybir
import concourse.tile as tile
from concourse import bacc, masks
from concourse.bass_utils import run_bass_kernel_spmd

F32 = mybir.dt.float32
BF16 = mybir.dt.bfloat16
I16 = mybir.dt.int16
AF = mybir.ActivationFunctionType
ALU = mybir.AluOpType

B, S, E, H = 4, 4096, 1024, 64
SH = S // 2           # queries per core
N_CORES = 8
ET = E // 128         # 8 embedding tiles
FC = 512              # projection chunk (cols of the seq axis)
NCH = S // FC         # 8 chunks over the full sequence
ST = S // 128         # 32 kj tiles over the full sequence
QC = 1024             # query chunk (one PSUM ctx tile)
SCALE = 0.125         # 1/sqrt(H)
# Schraudolph bf16 exp: i16 = round(128*log2(e)*(SCALE*s) + 16256 - 7.4)
SCH_A = 128.0 * 1.4426950408889634 * SCALE
SCH_B = 16256.0 - 7.4


def _emit(nc, tc, xt, wkv, wq, bkv, bq, wo, out_ext):
    with ExitStack() as top:
        const = top.enter_context(tc.tile_pool(name="const", bufs=1))

        ident = const.tile([128, 128], BF16)
        masks.make_identity(nc, ident[:])

        # Weights + biases on the gpsimd SWDGE queue; x gets both HWDGE
        # queues to itself so nothing delays the projection chunks.
        wkv_sb = const.tile([128, ET * 128], BF16)
        nc.gpsimd.dma_start(wkv_sb[:], wkv[:, :])
        wq_sb = const.tile([128, ET * 128], BF16)
        nc.gpsimd.dma_start(wq_sb[:], wq[:, :])
        bkv_sb = const.tile([128, 1], F32)
        nc.gpsimd.dma_start(bkv_sb[:], bkv.unsqueeze(1))
        bq_sb = const.tile([64, 1], F32)
        nc.gpsimd.dma_start(bq_sb[:], bq.unsqueeze(1))

        # Persistent operands. x_sb holds the 8 e-tiles side by side.
        x_sb = const.tile([128, ET * S], BF16)
        k2 = const.tile([128, S], BF16)     # kT on 0:64, zeros on 64:128
        q2 = const.tile([128, SH], BF16)    # qT on 0:64, zeros on 64:128
        vt_sb = const.tile([128, S], BF16)  # vT on rows 64:128 (PSUM-aligned)
        v_aug = const.tile([128, ST * 128], BF16)
        wo_sb = const.tile([128, E], BF16)
        ones11 = const.tile([1, 1], BF16)

        # x in 1024-col blocks x 8 e-slices, alternating between the two
        # HWDGE queues so descriptor generation runs in parallel.
        for blk in range(S // 1024):
            f0 = blk * 1024
            for e in range(ET):
                eng = nc.sync if e % 2 == 0 else nc.scalar
                eng.dma_start(
                    x_sb[:, e * S + f0 : e * S + f0 + 1024],
                    xt[e * 128 : (e + 1) * 128, f0 : f0 + 1024],
                )

        nc.gpsimd.memset(k2[64:128, :], 0.0)
        nc.gpsimd.memset(q2[64:128, :], 0.0)
        nc.gpsimd.memset(v_aug[:], 0.0)
        # ones column (index 64) of every kj tile; junk cols 65:128 stay 0
        nc.gpsimd.memset(
            v_aug[:].rearrange("p (t c) -> p t c", c=128)[:, :, 64:65], 1.0
        )
        nc.gpsimd.memset(ones11[:], 1.0)
        nc.gpsimd.dma_start(wo_sb[:], wo[:, :])  # needed only in phase C

        # ---- Phase A: QKV projection + V transpose ----------------------
        with ExitStack() as pa:
            mkvp = pa.enter_context(tc.tile_pool(name="mkv", bufs=2, space="PSUM"))
            mqp = pa.enter_context(tc.tile_pool(name="mq", bufs=2, space="PSUM"))
            vtp = pa.enter_context(tc.tile_pool(name="vtp", bufs=2, space="PSUM"))
            for c in range(NCH):
                f0 = c * FC
                mkv = mkvp.tile([128, FC], F32)
                for e in range(ET):
                    nc.tensor.matmul(
                        mkv[:],
                        wkv_sb[:, e * 128 : (e + 1) * 128],
                        x_sb[:, e * S + f0 : e * S + f0 + FC],
                        start=(e == 0), stop=(e == ET - 1),
                    )
                if c < 4:  # own half: also project Q
                    mq = mqp.tile([128, FC], F32)
                    for e in range(ET):
                        nc.tensor.matmul(
                            mq[:],
                            wq_sb[:, e * 128 : (e + 1) * 128],
                            x_sb[:, e * S + f0 : e * S + f0 + FC],
                            start=(e == 0), stop=(e == ET - 1),
                        )
                    nc.vector.tensor_scalar_add(
                        q2[0:64, f0 : f0 + FC], mq[0:64, :], bq_sb[:]
                    )
                nc.vector.tensor_scalar_add(
                    k2[0:64, f0 : f0 + FC], mkv[0:64, :], bkv_sb[0:64, :]
                )
                nc.vector.tensor_scalar_add(
                    vt_sb[64:128, f0 : f0 + FC], mkv[64:128, :], bkv_sb[64:128, :]
                )
                for t in range(FC // 128):
                    kj = c * (FC // 128) + t
                    vp = vtp.tile([128, 64], F32)
                    nc.tensor.matmul(
                        vp[:],
                        vt_sb[64:128, kj * 128 : (kj + 1) * 128],
                        ident[64:128, 64:128],
                    )
                    nc.vector.tensor_copy(v_aug[:, kj * 128 : kj * 128 + 64], vp[:])

        # ---- Phase B: scores -> exp -> ctx accumulation -----------------
        with ExitStack() as pbc:
            cps = pbc.enter_context(tc.tile_pool(name="cps", bufs=1, space="PSUM"))
            expp = pbc.enter_context(tc.tile_pool(name="expp", bufs=6))
            ctxs = [cps.tile([128, QC], F32, name=f"ctx{i}") for i in range(2)]

            with ExitStack() as pb:
                sps = pb.enter_context(tc.tile_pool(name="sps", bufs=2, space="PSUM"))
                for kj in range(ST):
                    lhs_k = k2[:, kj * 128 : (kj + 1) * 128]
                    lhs_v = v_aug[:, kj * 128 : (kj + 1) * 128]
                    for qix in range(2):
                        ctx = ctxs[qix]
                        q0 = qix * QC
                        sp = sps.tile([128, QC], F32)
                        for n in range(QC // 512):
                            nc.tensor.matmul(
                                sp[:, n * 512 : (n + 1) * 512],
                                lhs_k,
                                q2[:, q0 + n * 512 : q0 + (n + 1) * 512],
                            )
                        ex = expp.tile([128, QC], BF16)
                        # split the softmax exp: ACT gets 2 of every 3 tiles
                        # (exact), DVE the third (Schraudolph bf16 bitcast)
                        if (2 * kj + qix) % 3 == 2:
                            nc.vector.tensor_scalar(
                                ex[:].bitcast(I16),
                                sp[:],
                                SCH_A,
                                SCH_B,
                                op0=ALU.mult,
                                op1=ALU.add,
                            )
                        else:
                            nc.scalar.activation(ex[:], sp[:], AF.Exp, scale=SCALE)
                        for n in range(QC // 512):
                            nc.tensor.matmul(
                                ctx[:, n * 512 : (n + 1) * 512],
                                lhs_v,
                                ex[:, n * 512 : (n + 1) * 512],
                                start=(kj == 0), stop=(kj == ST - 1),
                                skip_group_check=True,
                            )

            # ---- Phase C: output projection -----------------------------
            with ExitStack() as pc:
                ops = pc.enter_context(tc.tile_pool(name="ops", bufs=2, space="PSUM"))
                rsps = pc.enter_context(tc.tile_pool(name="rsps", bufs=1, space="PSUM"))
                ctxp = pc.enter_context(tc.tile_pool(name="ctxp", bufs=2))
                rsp = pc.enter_context(tc.tile_pool(name="rsp", bufs=4))
                outp = pc.enter_context(tc.tile_pool(name="outp", bufs=4))

                ctx16s, recips = [], []
                # both qc chains emitted up front so their latencies overlap
                for qix in range(2):
                    ctx16 = ctxp.tile([128, QC], BF16, tag="ctx16")
                    # rows 65:128 are exact zeros (v_aug junk cols are 0)
                    nc.vector.tensor_copy(ctx16[:], ctxs[qix][:])
                    ctx16s.append(ctx16)
                    rs_row = rsp.tile([1, QC], BF16, tag="rsrow")
                    nc.sync.dma_start(rs_row[:], ctx16[64:65, :])
                    rs_ps = rsps.tile([128, QC // 128], F32, tag="rsps")
                    for cc in range(QC // 128):
                        nc.tensor.matmul(
                            rs_ps[:, cc : cc + 1],
                            rs_row[0:1, cc * 128 : (cc + 1) * 128],
                            ones11[:],
                        )
                    recip = rsp.tile([128, QC // 128], F32, tag="recip")
                    nc.vector.reciprocal(recip[:], rs_ps[:])
                    recips.append(recip)

                for cc in range(QC // 128):
                    for qix in range(2):
                        ctx16, recip = ctx16s[qix], recips[qix]
                        q0 = qix * QC
                        out_sb = outp.tile([128, E], BF16)
                        for n in range(2):
                            op = ops.tile([128, 512], F32)
                            nc.tensor.matmul(
                                op[:],
                                ctx16[:, cc * 128 : (cc + 1) * 128],
                                wo_sb[:, n * 512 : (n + 1) * 512],
                            )
                            # Pool can't read PSUM; ACT is idle here, so
                            # alternate the recip-scale between DVE and ACT
                            if (cc + n + qix) % 2 == 0:
                                nc.vector.tensor_scalar_mul(
                                    out_sb[:, n * 512 : (n + 1) * 512],
                                    op[:],
                                    recip[:, cc : cc + 1],
                                )
                            else:
                                nc.scalar.mul(
                                    out_sb[:, n * 512 : (n + 1) * 512],
                                    op[:],
                                    recip[:, cc : cc + 1],
                                )
                        nc.sync.dma_start(
                            out_ext[q0 + cc * 128 : q0 + (cc + 1) * 128, :],
                            out_sb[:],
                        )


_NC = None


def _get_nc():
    global _NC
    if _NC is None:
        nc = bacc.Bacc("TRN2", target_bir_lowering=False, debug=False,
                       num_devices=N_CORES)
        xt = nc.dram_tensor("xt", [E, S], BF16, kind="ExternalInput").ap()
        wkv = nc.dram_tensor("wkv", [128, ET * 128], BF16, kind="ExternalInput").ap()
        wq = nc.dram_tensor("wq", [128, ET * 128], BF16, kind="ExternalInput").ap()
        bkv = nc.dram_tensor("bkv", [128], F32, kind="ExternalInput").ap()
        bq = nc.dram_tensor("bq", [64], F32, kind="ExternalInput").ap()
        wo = nc.dram_tensor("wo", [128, E], BF16, kind="ExternalInput").ap()
        out_ext = nc.dram_tensor("out", [SH, E], BF16, kind="ExternalOutput").ap()
        with tile.TileContext(nc) as tc:
            _emit(nc, tc, xt, wkv, wq, bkv, bq, wo, out_ext)
        nc.compile()
        _NC = nc
    return _NC


last_results = None
last_tmpdir = None


def kernel(x, W_qkv, b_qkv, W_out, b_out):
    nc = _get_nc()
    bf = ml_dtypes.bfloat16
    x = np.asarray(x, dtype=np.float32)
    Wq = np.asarray(W_qkv, dtype=np.float32)
    b1 = np.asarray(b_qkv, dtype=np.float32)

    wkv = np.empty((128, ET * 128), dtype=bf)
    wq_p = np.zeros((128, ET * 128), dtype=bf)
    for e in range(ET):
        wkv[:, e * 128 : e * 128 + 64] = Wq[e * 128 : (e + 1) * 128, 64:128]
        wkv[:, e * 128 + 64 : (e + 1) * 128] = Wq[e * 128 : (e + 1) * 128, 128:192]
        wq_p[:, e * 128 : e * 128 + 64] = Wq[e * 128 : (e + 1) * 128, 0:64]
    bkv = np.concatenate([b1[64:128], b1[128:192]]).astype(np.float32)
    bq = np.ascontiguousarray(b1[0:64])
    wo = np.zeros((128, E), dtype=bf)
    wo[0:64] = np.asarray(W_out, dtype=np.float32)
    wo[64] = np.asarray(b_out, dtype=np.float32)

    shared = {"wkv": wkv, "wq": wq_p, "bkv": bkv, "bq": bq, "wo": wo}
    in_maps = []
    for c in range(N_CORES):
        b, h = divmod(c, 2)
        xb = x[b]
        xt = np.empty((E, S), dtype=bf)
        xt[:, 0:SH] = xb[h * SH : (h + 1) * SH].T
        xt[:, SH:S] = xb[(1 - h) * SH : (2 - h) * SH].T
        in_maps.append({"xt": xt, **shared})

    import os
    import tempfile
    import time

    tmpdir = os.environ.get("ATTN_TRACE_DIR") or tempfile.mkdtemp(prefix="attn_trace_")
    res = None
    for attempt in range(3):
        try:
            res = run_bass_kernel_spmd(
                nc, in_maps, core_ids=list(range(N_CORES)), tmpdir=tmpdir
            )
            break
        except Exception:
            # transient NRT_EXEC_UNIT_UNRECOVERABLE has been observed on a
            # first attempt; a clean retry recovers
            if attempt == 2:
                raise
            time.sleep(2.0)
    global last_results, last_tmpdir
    last_results = res
    last_tmpdir = tmpdir

    out = np.empty((B, S, E), dtype=np.float32)
    for c in range(N_CORES):
        b, h = divmod(c, 2)
        out[b, h * SH : (h + 1) * SH] = res.results[c]["out"].astype(np.float32)
    return out


# revision 7
# speedup vs baseline: 1.0799x; 1.0754x over previous
"""Single-head attention (B=4, S=4096, E=1024, H=64) on 8 TRN2 NeuronCores.

Sharding: core c -> (batch b = c//2, sequence half h = c%2). No collectives:
each core receives the transposed bf16 x for its WHOLE batch row, laid out
own-half-first, computes K/V for the full 4096-key sequence plus Q for its
own 2048 queries, then runs attention and the output projection for its
queries. Softmax over keys is permutation invariant, so the own-first key
order needs no unpermute.

Matmuls are bf16 (fp8 was measured numerically dead for this problem: the
softmax is extremely peaked, Neff ~ 6, so e4m3 noise doesn't average out).
All large matmuls use full 128x128 stationary tiles (zero/junk padded) to
keep the PE at speed -- masked sub-tiles clock-gate the PE.

Softmax exp is split across two engines: the ACT engine computes exact Exp
for ~2/3 of the score tiles, and the DVE computes the rest with a
bf16 Schraudolph approximation (i16 = 23.083*s + 16249; bitcast to bf16
is 2^(logit*log2e) with ~2.3% sawtooth error; measured end-to-end rel err
~8e-3, budget 2e-2). x input lands via two HWDGE queues (sync + scalar)
to halve the descriptor-generation serialization.

Output projection uses the augmented-row trick: W_out carries b_out as row
64 and the bf16 context carries the softmax denominator in row 64, so
(ctx_aug.T @ W_out_aug) * recip(denom) applies scale and bias in one pass
(denom * recip == 1)."""

import sys

import numpy as np

for _p in ("/opt/trn_rl_repo",):
    if _p not in sys.path:
        sys.path.insert(0, _p)

from contextlib import ExitStack

import ml_dtypes

import concourse.bass as bass  # noqa: F401  (import keeps bass registered)
import concourse.mybir as mybir
import concourse.tile as tile
from concourse import bacc, masks
from concourse.bass_utils import run_bass_kernel_spmd

F32 = mybir.dt.float32
BF16 = mybir.dt.bfloat16
I16 = mybir.dt.int16
AF = mybir.ActivationFunctionType
ALU = mybir.AluOpType

B, S, E, H = 4, 4096, 1024, 64
SH = S // 2           # queries per core
N_CORES = 8
ET = E // 128         # 8 embedding tiles
FC = 512              # projection chunk (cols of the seq axis)
NCH = S // FC         # 8 chunks over the full sequence
ST = S // 128         # 32 kj tiles over the full sequence
QC = 1024             # query chunk (one PSUM ctx tile)
SCALE = 0.125         # 1/sqrt(H)
# Schraudolph bf16 exp: i16 = round(128*log2(e)*(SCALE*s) + 16256 - 7.4)
SCH_A = 128.0 * 1.4426950408889634 * SCALE
SCH_B = 16256.0 - 7.4


def _emit(nc, tc, xt, wkv, wq, bkv, bq, wo, out_ext):
    with ExitStack() as top:
        const = top.enter_context(tc.tile_pool(name="const", bufs=1))

        ident = const.tile([128, 128], BF16)
        masks.make_identity(nc, ident[:])

        # Weights + biases on the gpsimd SWDGE queue; x gets both HWDGE
        # queues to itself so nothing delays the projection chunks.
        wkv_sb = const.tile([128, ET * 128], BF16)
        nc.gpsimd.dma_start(wkv_sb[:], wkv[:, :])
        wq_sb = const.tile([128, ET * 128], BF16)
        nc.gpsimd.dma_start(wq_sb[:], wq[:, :])
        bkv_sb = const.tile([128, 1], F32)
        nc.gpsimd.dma_start(bkv_sb[:], bkv.unsqueeze(1))
        bq_sb = const.tile([64, 1], F32)
        nc.gpsimd.dma_start(bq_sb[:], bq.unsqueeze(1))

        # Persistent operands. x_sb holds the 8 e-tiles side by side.
        x_sb = const.tile([128, ET * S], BF16)
        k2 = const.tile([128, S], BF16)     # kT on 0:64, zeros on 64:128
        q2 = const.tile([128, SH], BF16)    # qT on 0:64, zeros on 64:128
        vt_sb = const.tile([128, S], BF16)  # vT on rows 64:128 (PSUM-aligned)
        v_aug = const.tile([128, ST * 128], BF16)
        wo_sb = const.tile([128, E], BF16)
        ones11 = const.tile([1, 1], BF16)

        # x in 2048-col blocks x 8 e-slices, alternating between the two
        # HWDGE queues: descriptor generation (~0.6us per DMA regardless of
        # size) runs in parallel and fewer triggers start the transfers
        # sooner.
        for blk in range(S // 2048):
            f0 = blk * 2048
            for e in range(ET):
                eng = nc.sync if e % 2 == 0 else nc.scalar
                eng.dma_start(
                    x_sb[:, e * S + f0 : e * S + f0 + 2048],
                    xt[e * 128 : (e + 1) * 128, f0 : f0 + 2048],
                )

        # memsets on DVE (idle until the first projection chunk lands);
        # only the ones column + junk cols of v_aug need initialization
        nc.vector.memset(k2[64:128, :], 0.0)
        nc.vector.memset(q2[64:128, :], 0.0)
        v_aug_t = v_aug[:].rearrange("p (t c) -> p t c", c=128)
        nc.vector.memset(v_aug_t[:, :, 65:128], 0.0)
        nc.vector.memset(v_aug_t[:, :, 64:65], 1.0)
        nc.vector.memset(ones11[:], 1.0)
        nc.gpsimd.dma_start(wo_sb[:], wo[:, :])  # needed only in phase C

        # ---- Phase A: QKV projection + V transpose ----------------------
        with ExitStack() as pa:
            mkvp = pa.enter_context(tc.tile_pool(name="mkv", bufs=2, space="PSUM"))
            mqp = pa.enter_context(tc.tile_pool(name="mq", bufs=2, space="PSUM"))
            vtp = pa.enter_context(tc.tile_pool(name="vtp", bufs=2, space="PSUM"))
            for c in range(NCH):
                f0 = c * FC
                mkv = mkvp.tile([128, FC], F32)
                for e in range(ET):
                    nc.tensor.matmul(
                        mkv[:],
                        wkv_sb[:, e * 128 : (e + 1) * 128],
                        x_sb[:, e * S + f0 : e * S + f0 + FC],
                        start=(e == 0), stop=(e == ET - 1),
                    )
                if c < 4:  # own half: also project Q
                    mq = mqp.tile([128, FC], F32)
                    for e in range(ET):
                        nc.tensor.matmul(
                            mq[:],
                            wq_sb[:, e * 128 : (e + 1) * 128],
                            x_sb[:, e * S + f0 : e * S + f0 + FC],
                            start=(e == 0), stop=(e == ET - 1),
                        )
                    nc.vector.tensor_scalar_add(
                        q2[0:64, f0 : f0 + FC], mq[0:64, :], bq_sb[:]
                    )
                nc.vector.tensor_scalar_add(
                    k2[0:64, f0 : f0 + FC], mkv[0:64, :], bkv_sb[0:64, :]
                )
                nc.vector.tensor_scalar_add(
                    vt_sb[64:128, f0 : f0 + FC], mkv[64:128, :], bkv_sb[64:128, :]
                )
                for t in range(FC // 128):
                    kj = c * (FC // 128) + t
                    vp = vtp.tile([128, 64], F32)
                    nc.tensor.matmul(
                        vp[:],
                        vt_sb[64:128, kj * 128 : (kj + 1) * 128],
                        ident[64:128, 64:128],
                    )
                    nc.vector.tensor_copy(v_aug[:, kj * 128 : kj * 128 + 64], vp[:])

        # ---- Phase B: scores -> exp -> ctx accumulation -----------------
        with ExitStack() as pbc:
            cps = pbc.enter_context(tc.tile_pool(name="cps", bufs=1, space="PSUM"))
            expp = pbc.enter_context(tc.tile_pool(name="expp", bufs=6))
            ctxs = [cps.tile([128, QC], F32, name=f"ctx{i}") for i in range(2)]

            with ExitStack() as pb:
                sps = pb.enter_context(tc.tile_pool(name="sps", bufs=2, space="PSUM"))

                def emit_ctx(kj, exs):
                    lhs_v = v_aug[:, kj * 128 : (kj + 1) * 128]
                    for qix in range(2):
                        for n in range(QC // 512):
                            nc.tensor.matmul(
                                ctxs[qix][:, n * 512 : (n + 1) * 512],
                                lhs_v,
                                exs[qix][:, n * 512 : (n + 1) * 512],
                                start=(kj == 0), stop=(kj == ST - 1),
                                skip_group_check=True,
                            )

                # software-pipelined: ctx(kj-1) is emitted after sc/exp(kj)
                # so the in-order PE queue never stalls on an exp result
                pend = None
                for kj in range(ST):
                    lhs_k = k2[:, kj * 128 : (kj + 1) * 128]
                    exs = []
                    for qix in range(2):
                        q0 = qix * QC
                        sp = sps.tile([128, QC], F32)
                        for n in range(QC // 512):
                            nc.tensor.matmul(
                                sp[:, n * 512 : (n + 1) * 512],
                                lhs_k,
                                q2[:, q0 + n * 512 : q0 + (n + 1) * 512],
                            )
                        ex = expp.tile([128, QC], BF16)
                        # split the softmax exp: ACT gets 2 of every 3 tiles
                        # (exact), DVE the third (Schraudolph bf16 bitcast)
                        if (2 * kj + qix) % 3 == 2:
                            nc.vector.tensor_scalar(
                                ex[:].bitcast(I16),
                                sp[:],
                                SCH_A,
                                SCH_B,
                                op0=ALU.mult,
                                op1=ALU.add,
                            )
                        else:
                            nc.scalar.activation(ex[:], sp[:], AF.Exp, scale=SCALE)
                        exs.append(ex)
                    if pend is not None:
                        emit_ctx(*pend)
                    pend = (kj, exs)
                emit_ctx(*pend)

            # ---- Phase C: output projection -----------------------------
            with ExitStack() as pc:
                ops = pc.enter_context(tc.tile_pool(name="ops", bufs=2, space="PSUM"))
                rsps = pc.enter_context(tc.tile_pool(name="rsps", bufs=1, space="PSUM"))
                ctxp = pc.enter_context(tc.tile_pool(name="ctxp", bufs=2))
                rsp = pc.enter_context(tc.tile_pool(name="rsp", bufs=4))
                outp = pc.enter_context(tc.tile_pool(name="outp", bufs=4))

                ctx16s, recips = [], []
                # both qc chains emitted up front so their latencies overlap
                for qix in range(2):
                    ctx16 = ctxp.tile([128, QC], BF16, tag="ctx16")
                    # rows 65:128 are exact zeros (v_aug junk cols are 0)
                    nc.vector.tensor_copy(ctx16[:], ctxs[qix][:])
                    ctx16s.append(ctx16)
                    rs_row = rsp.tile([1, QC], BF16, tag="rsrow")
                    nc.sync.dma_start(rs_row[:], ctx16[64:65, :])
                    rs_ps = rsps.tile([128, QC // 128], F32, tag="rsps")
                    for cc in range(QC // 128):
                        nc.tensor.matmul(
                            rs_ps[:, cc : cc + 1],
                            rs_row[0:1, cc * 128 : (cc + 1) * 128],
                            ones11[:],
                        )
                    recip = rsp.tile([128, QC // 128], F32, tag="recip")
                    nc.vector.reciprocal(recip[:], rs_ps[:])
                    recips.append(recip)

                for cc in range(QC // 128):
                    for qix in range(2):
                        ctx16, recip = ctx16s[qix], recips[qix]
                        q0 = qix * QC
                        out_sb = outp.tile([128, E], BF16)
                        for n in range(2):
                            op = ops.tile([128, 512], F32)
                            nc.tensor.matmul(
                                op[:],
                                ctx16[:, cc * 128 : (cc + 1) * 128],
                                wo_sb[:, n * 512 : (n + 1) * 512],
                            )
                            # Pool can't read PSUM; ACT is idle here, so
                            # alternate the recip-scale between DVE and ACT
                            if (cc + n + qix) % 2 == 0:
                                nc.vector.tensor_scalar_mul(
                                    out_sb[:, n * 512 : (n + 1) * 512],
                                    op[:],
                                    recip[:, cc : cc + 1],
                                )
                            else:
                                nc.scalar.mul(
                                    out_sb[:, n * 512 : (n + 1) * 512],
                                    op[:],
                                    recip[:, cc : cc + 1],
                                )
                        nc.sync.dma_start(
                            out_ext[q0 + cc * 128 : q0 + (cc + 1) * 128, :],
                            out_sb[:],
                        )


_NC = None


def _get_nc():
    global _NC
    if _NC is None:
        nc = bacc.Bacc("TRN2", target_bir_lowering=False, debug=False,
                       num_devices=N_CORES)
        xt = nc.dram_tensor("xt", [E, S], BF16, kind="ExternalInput").ap()
        wkv = nc.dram_tensor("wkv", [128, ET * 128], BF16, kind="ExternalInput").ap()
        wq = nc.dram_tensor("wq", [128, ET * 128], BF16, kind="ExternalInput").ap()
        bkv = nc.dram_tensor("bkv", [128], F32, kind="ExternalInput").ap()
        bq = nc.dram_tensor("bq", [64], F32, kind="ExternalInput").ap()
        wo = nc.dram_tensor("wo", [128, E], BF16, kind="ExternalInput").ap()
        out_ext = nc.dram_tensor("out", [SH, E], BF16, kind="ExternalOutput").ap()
        with tile.TileContext(nc) as tc:
            _emit(nc, tc, xt, wkv, wq, bkv, bq, wo, out_ext)
        nc.compile()
        _NC = nc
    return _NC


last_results = None
last_tmpdir = None


def kernel(x, W_qkv, b_qkv, W_out, b_out):
    nc = _get_nc()
    bf = ml_dtypes.bfloat16
    x = np.asarray(x, dtype=np.float32)
    Wq = np.asarray(W_qkv, dtype=np.float32)
    b1 = np.asarray(b_qkv, dtype=np.float32)

    wkv = np.empty((128, ET * 128), dtype=bf)
    wq_p = np.zeros((128, ET * 128), dtype=bf)
    for e in range(ET):
        wkv[:, e * 128 : e * 128 + 64] = Wq[e * 128 : (e + 1) * 128, 64:128]
        wkv[:, e * 128 + 64 : (e + 1) * 128] = Wq[e * 128 : (e + 1) * 128, 128:192]
        wq_p[:, e * 128 : e * 128 + 64] = Wq[e * 128 : (e + 1) * 128, 0:64]
    bkv = np.concatenate([b1[64:128], b1[128:192]]).astype(np.float32)
    bq = np.ascontiguousarray(b1[0:64])
    wo = np.zeros((128, E), dtype=bf)
    wo[0:64] = np.asarray(W_out, dtype=np.float32)
    wo[64] = np.asarray(b_out, dtype=np.float32)

    shared = {"wkv": wkv, "wq": wq_p, "bkv": bkv, "bq": bq, "wo": wo}
    in_maps = []
    for c in range(N_CORES):
        b, h = divmod(c, 2)
        xb = x[b]
        xt = np.empty((E, S), dtype=bf)
        xt[:, 0:SH] = xb[h * SH : (h + 1) * SH].T
        xt[:, SH:S] = xb[(1 - h) * SH : (2 - h) * SH].T
        in_maps.append({"xt": xt, **shared})

    import os
    import tempfile
    import time

    tmpdir = os.environ.get("ATTN_TRACE_DIR") or tempfile.mkdtemp(prefix="attn_trace_")
    res = None
    for attempt in range(3):
        try:
            res = run_bass_kernel_spmd(
                nc, in_maps, core_ids=list(range(N_CORES)), tmpdir=tmpdir
            )
            break
        except Exception:
            # transient NRT_EXEC_UNIT_UNRECOVERABLE has been observed on a
            # first attempt; a clean retry recovers
            if attempt == 2:
                raise
            time.sleep(2.0)
    global last_results, last_tmpdir
    last_results = res
    last_tmpdir = tmpdir

    out = np.empty((B, S, E), dtype=np.float32)
    for c in range(N_CORES):
        b, h = divmod(c, 2)
        out[b, h * SH : (h + 1) * SH] = res.results[c]["out"].astype(np.float32)
    return out


# revision 9
# speedup vs baseline: 1.1371x; 1.0529x over previous
"""Single-head attention (B=4, S=4096, E=1024, H=64) on 8 TRN2 NeuronCores.

Sharding: core c -> (batch b = c//2, sequence half h = c%2). No collectives:
each core receives the transposed bf16 x for its WHOLE batch row, laid out
own-half-first, computes K/V for the full 4096-key sequence plus Q for its
own 2048 queries, then runs attention and the output projection for its
queries. Softmax over keys is permutation invariant, so the own-first key
order needs no unpermute.

Matmuls are bf16 (fp8 was measured numerically dead for this problem: the
softmax is extremely peaked, Neff ~ 6, so e4m3 noise doesn't average out).
All large matmuls use full 128x128 stationary tiles (zero/junk padded) to
keep the PE at speed -- masked sub-tiles clock-gate the PE.

Softmax exp is split across two engines: the ACT engine computes exact Exp
for ~2/3 of the score tiles, and the DVE computes the rest with a
bf16 Schraudolph approximation (i16 = 23.083*s + 16249; bitcast to bf16
is 2^(logit*log2e) with ~2.3% sawtooth error; measured end-to-end rel err
~8e-3, budget 2e-2). x input lands via two HWDGE queues (sync + scalar)
to halve the descriptor-generation serialization.

Output projection uses the augmented-row trick: W_out carries b_out as row
64 and the bf16 context carries the softmax denominator in row 64, so
(ctx_aug.T @ W_out_aug) * recip(denom) applies scale and bias in one pass
(denom * recip == 1)."""

import sys

import numpy as np

for _p in ("/opt/trn_rl_repo",):
    if _p not in sys.path:
        sys.path.insert(0, _p)

from contextlib import ExitStack

import ml_dtypes

import concourse.bass as bass  # noqa: F401  (import keeps bass registered)
import concourse.mybir as mybir
import concourse.tile as tile
from concourse import bacc, masks
from concourse.bass_utils import run_bass_kernel_spmd

F32 = mybir.dt.float32
BF16 = mybir.dt.bfloat16
I16 = mybir.dt.int16
AF = mybir.ActivationFunctionType
ALU = mybir.AluOpType

B, S, E, H = 4, 4096, 1024, 64
SH = S // 2           # queries per core
N_CORES = 8
ET = E // 128         # 8 embedding tiles
FC = 512              # projection chunk (cols of the seq axis)
NCH = S // FC         # 8 chunks over the full sequence
ST = S // 128         # 32 kj tiles over the full sequence
QC = 1024             # query chunk (one PSUM ctx tile)
SCALE = 0.125         # 1/sqrt(H)
# Schraudolph bf16 exp: i16 = round(128*log2(e)*(SCALE*s) + 16256 - 7.4)
SCH_A = 128.0 * 1.4426950408889634 * SCALE
SCH_B = 16256.0 - 7.4


def _emit(nc, tc, xt, wkv, wq, bkv, bq, wo, out_ext):
    with ExitStack() as top:
        const = top.enter_context(tc.tile_pool(name="const", bufs=1))

        ident = const.tile([128, 128], BF16)
        masks.make_identity(nc, ident[:])

        # Weights + biases on the gpsimd SWDGE queue; x gets both HWDGE
        # queues to itself so nothing delays the projection chunks.
        wkv_sb = const.tile([128, ET * 128], BF16)
        nc.gpsimd.dma_start(wkv_sb[:], wkv[:, :])
        wq_sb = const.tile([128, ET * 128], BF16)
        nc.gpsimd.dma_start(wq_sb[:], wq[:, :])
        bkv_sb = const.tile([128, 1], F32)
        nc.gpsimd.dma_start(bkv_sb[:], bkv.unsqueeze(1))
        bq_sb = const.tile([64, 1], F32)
        nc.gpsimd.dma_start(bq_sb[:], bq.unsqueeze(1))

        # Persistent operands. x_sb holds the 8 e-tiles side by side.
        x_sb = const.tile([128, ET * S], BF16)
        k2 = const.tile([128, S], BF16)     # kT on 0:64, zeros on 64:128
        q2 = const.tile([128, SH], BF16)    # qT on 0:64, zeros on 64:128
        vt_sb = const.tile([128, S], BF16)  # vT on rows 64:128 (PSUM-aligned)
        v_aug = const.tile([128, ST * 128], BF16)
        wo_sb = const.tile([128, E], BF16)
        ones11 = const.tile([1, 1], BF16)

        # x over both HWDGE queues. Narrow first blocks so the first
        # projection chunk starts ASAP (per-queue transfers serialize, so a
        # 2048-wide first block would delay chunk 0 by ~7us); wide later
        # blocks amortize the ~0.6us descriptor generation per DMA.
        for f0, w in ((0, 512), (512, 512), (1024, 1024), (2048, 2048)):
            for e in range(ET):
                eng = nc.sync if e % 2 == 0 else nc.scalar
                eng.dma_start(
                    x_sb[:, e * S + f0 : e * S + f0 + w],
                    xt[e * 128 : (e + 1) * 128, f0 : f0 + w],
                )

        # memsets on DVE (idle until the first projection chunk lands);
        # only the ones column + junk cols of v_aug need initialization
        nc.vector.memset(k2[64:128, :], 0.0)
        nc.vector.memset(q2[64:128, :], 0.0)
        v_aug_t = v_aug[:].rearrange("p (t c) -> p t c", c=128)
        nc.vector.memset(v_aug_t[:, :, 65:128], 0.0)
        nc.vector.memset(v_aug_t[:, :, 64:65], 1.0)
        nc.vector.memset(ones11[:], 1.0)
        nc.gpsimd.dma_start(wo_sb[:], wo[:, :])  # needed only in phase C

        # ---- Phase A: QKV projection + V transpose ----------------------
        with ExitStack() as pa:
            mkvp = pa.enter_context(tc.tile_pool(name="mkv", bufs=2, space="PSUM"))
            mqp = pa.enter_context(tc.tile_pool(name="mq", bufs=2, space="PSUM"))
            vtp = pa.enter_context(tc.tile_pool(name="vtp", bufs=2, space="PSUM"))
            for c in range(NCH):
                f0 = c * FC
                mkv = mkvp.tile([128, FC], F32)
                for e in range(ET):
                    nc.tensor.matmul(
                        mkv[:],
                        wkv_sb[:, e * 128 : (e + 1) * 128],
                        x_sb[:, e * S + f0 : e * S + f0 + FC],
                        start=(e == 0), stop=(e == ET - 1),
                    )
                if c < 4:  # own half: also project Q
                    mq = mqp.tile([128, FC], F32)
                    for e in range(ET):
                        nc.tensor.matmul(
                            mq[:],
                            wq_sb[:, e * 128 : (e + 1) * 128],
                            x_sb[:, e * S + f0 : e * S + f0 + FC],
                            start=(e == 0), stop=(e == ET - 1),
                        )
                    nc.vector.tensor_scalar_add(
                        q2[0:64, f0 : f0 + FC], mq[0:64, :], bq_sb[:]
                    )
                nc.vector.tensor_scalar_add(
                    k2[0:64, f0 : f0 + FC], mkv[0:64, :], bkv_sb[0:64, :]
                )
                nc.vector.tensor_scalar_add(
                    vt_sb[64:128, f0 : f0 + FC], mkv[64:128, :], bkv_sb[64:128, :]
                )
                for t in range(FC // 128):
                    kj = c * (FC // 128) + t
                    vp = vtp.tile([128, 64], F32)
                    nc.tensor.matmul(
                        vp[:],
                        vt_sb[64:128, kj * 128 : (kj + 1) * 128],
                        ident[64:128, 64:128],
                    )
                    nc.vector.tensor_copy(v_aug[:, kj * 128 : kj * 128 + 64], vp[:])

        # ---- Phase B: scores -> exp -> ctx accumulation -----------------
        with ExitStack() as pbc:
            cps = pbc.enter_context(tc.tile_pool(name="cps", bufs=1, space="PSUM"))
            expp = pbc.enter_context(tc.tile_pool(name="expp", bufs=6))
            ctxs = [cps.tile([128, QC], F32, name=f"ctx{i}") for i in range(2)]

            with ExitStack() as pb:
                sps = pb.enter_context(tc.tile_pool(name="sps", bufs=2, space="PSUM"))

                def emit_ctx(kj, exs):
                    lhs_v = v_aug[:, kj * 128 : (kj + 1) * 128]
                    for qix in range(2):
                        for n in range(QC // 512):
                            nc.tensor.matmul(
                                ctxs[qix][:, n * 512 : (n + 1) * 512],
                                lhs_v,
                                exs[qix][:, n * 512 : (n + 1) * 512],
                                start=(kj == 0), stop=(kj == ST - 1),
                                skip_group_check=True,
                            )

                # software-pipelined: ctx(kj-1) is emitted after sc/exp(kj)
                # so the in-order PE queue never stalls on an exp result
                pend = None
                for kj in range(ST):
                    lhs_k = k2[:, kj * 128 : (kj + 1) * 128]
                    exs = []
                    for qix in range(2):
                        q0 = qix * QC
                        sp = sps.tile([128, QC], F32)
                        for n in range(QC // 512):
                            nc.tensor.matmul(
                                sp[:, n * 512 : (n + 1) * 512],
                                lhs_k,
                                q2[:, q0 + n * 512 : q0 + (n + 1) * 512],
                            )
                        ex = expp.tile([128, QC], BF16)
                        # split the softmax exp: ACT gets 2 of every 3 tiles
                        # (exact), DVE the third (Schraudolph bf16 bitcast)
                        if (2 * kj + qix) % 3 == 2:
                            nc.vector.tensor_scalar(
                                ex[:].bitcast(I16),
                                sp[:],
                                SCH_A,
                                SCH_B,
                                op0=ALU.mult,
                                op1=ALU.add,
                            )
                        else:
                            nc.scalar.activation(ex[:], sp[:], AF.Exp, scale=SCALE)
                        exs.append(ex)
                    if pend is not None:
                        emit_ctx(*pend)
                    pend = (kj, exs)
                emit_ctx(*pend)

            # ---- Phase C: output projection -----------------------------
            with ExitStack() as pc:
                ops = pc.enter_context(tc.tile_pool(name="ops", bufs=3, space="PSUM"))
                rsps = pc.enter_context(tc.tile_pool(name="rsps", bufs=1, space="PSUM"))
                ctxp = pc.enter_context(tc.tile_pool(name="ctxp", bufs=2))
                rsp = pc.enter_context(tc.tile_pool(name="rsp", bufs=4))
                outp = pc.enter_context(tc.tile_pool(name="outp", bufs=6))

                ctx16s, recips = [], []
                # both qc chains emitted up front so their latencies overlap
                for qix in range(2):
                    ctx16 = ctxp.tile([128, QC], BF16, tag="ctx16")
                    # rows 65:128 are exact zeros (v_aug junk cols are 0)
                    nc.vector.tensor_copy(ctx16[:], ctxs[qix][:])
                    ctx16s.append(ctx16)
                    rs_row = rsp.tile([1, QC], BF16, tag="rsrow")
                    nc.sync.dma_start(rs_row[:], ctx16[64:65, :])
                    rs_ps = rsps.tile([128, QC // 128], F32, tag="rsps")
                    for cc in range(QC // 128):
                        nc.tensor.matmul(
                            rs_ps[:, cc : cc + 1],
                            rs_row[0:1, cc * 128 : (cc + 1) * 128],
                            ones11[:],
                        )
                    recip = rsp.tile([128, QC // 128], F32, tag="recip")
                    nc.vector.reciprocal(recip[:], rs_ps[:])
                    recips.append(recip)

                # matmuls don't need recip -- emit them densely (keeps the
                # PE streaming so HAM holds full clock); scale-muls trail
                pend_mul = []
                for cc in range(QC // 128):
                    for qix in range(2):
                        ctx16, recip = ctx16s[qix], recips[qix]
                        out_sb = outp.tile([128, E], BF16)
                        for n in range(2):
                            op = ops.tile([128, 512], F32)
                            nc.tensor.matmul(
                                op[:],
                                ctx16[:, cc * 128 : (cc + 1) * 128],
                                wo_sb[:, n * 512 : (n + 1) * 512],
                            )
                            pend_mul.append((op, out_sb, qix, cc, n))
                        while len(pend_mul) > 2:
                            _emit_mul(nc, out_ext, recips, pend_mul.pop(0))
                while pend_mul:
                    _emit_mul(nc, out_ext, recips, pend_mul.pop(0))


def _emit_mul(nc, out_ext, recips, item):
    op, out_sb, qix, cc, n = item
    # Pool can't read PSUM; ACT is idle in phase C, so the recip-scale
    # alternates between DVE and ACT
    if (cc + n + qix) % 2 == 0:
        nc.vector.tensor_scalar_mul(
            out_sb[:, n * 512 : (n + 1) * 512], op[:], recips[qix][:, cc : cc + 1]
        )
    else:
        nc.scalar.mul(
            out_sb[:, n * 512 : (n + 1) * 512], op[:], recips[qix][:, cc : cc + 1]
        )
    if n == 1:
        nc.sync.dma_start(
            out_ext[qix * QC + cc * 128 : qix * QC + (cc + 1) * 128, :], out_sb[:]
        )


_NC = None


def _get_nc():
    global _NC
    if _NC is None:
        nc = bacc.Bacc("TRN2", target_bir_lowering=False, debug=False,
                       num_devices=N_CORES)
        xt = nc.dram_tensor("xt", [E, S], BF16, kind="ExternalInput").ap()
        wkv = nc.dram_tensor("wkv", [128, ET * 128], BF16, kind="ExternalInput").ap()
        wq = nc.dram_tensor("wq", [128, ET * 128], BF16, kind="ExternalInput").ap()
        bkv = nc.dram_tensor("bkv", [128], F32, kind="ExternalInput").ap()
        bq = nc.dram_tensor("bq", [64], F32, kind="ExternalInput").ap()
        wo = nc.dram_tensor("wo", [128, E], BF16, kind="ExternalInput").ap()
        out_ext = nc.dram_tensor("out", [SH, E], BF16, kind="ExternalOutput").ap()
        with tile.TileContext(nc) as tc:
            _emit(nc, tc, xt, wkv, wq, bkv, bq, wo, out_ext)
        nc.compile()
        _NC = nc
    return _NC


last_results = None
last_tmpdir = None


def kernel(x, W_qkv, b_qkv, W_out, b_out):
    nc = _get_nc()
    bf = ml_dtypes.bfloat16
    x = np.asarray(x, dtype=np.float32)
    Wq = np.asarray(W_qkv, dtype=np.float32)
    b1 = np.asarray(b_qkv, dtype=np.float32)

    wkv = np.empty((128, ET * 128), dtype=bf)
    wq_p = np.zeros((128, ET * 128), dtype=bf)
    for e in range(ET):
        wkv[:, e * 128 : e * 128 + 64] = Wq[e * 128 : (e + 1) * 128, 64:128]
        wkv[:, e * 128 + 64 : (e + 1) * 128] = Wq[e * 128 : (e + 1) * 128, 128:192]
        wq_p[:, e * 128 : e * 128 + 64] = Wq[e * 128 : (e + 1) * 128, 0:64]
    bkv = np.concatenate([b1[64:128], b1[128:192]]).astype(np.float32)
    bq = np.ascontiguousarray(b1[0:64])
    wo = np.zeros((128, E), dtype=bf)
    wo[0:64] = np.asarray(W_out, dtype=np.float32)
    wo[64] = np.asarray(b_out, dtype=np.float32)

    shared = {"wkv": wkv, "wq": wq_p, "bkv": bkv, "bq": bq, "wo": wo}
    in_maps = []
    for c in range(N_CORES):
        b, h = divmod(c, 2)
        xb = x[b]
        xt = np.empty((E, S), dtype=bf)
        xt[:, 0:SH] = xb[h * SH : (h + 1) * SH].T
        xt[:, SH:S] = xb[(1 - h) * SH : (2 - h) * SH].T
        in_maps.append({"xt": xt, **shared})

    import os
    import tempfile
    import time

    tmpdir = os.environ.get("ATTN_TRACE_DIR") or tempfile.mkdtemp(prefix="attn_trace_")
    res = None
    for attempt in range(3):
        try:
            res = run_bass_kernel_spmd(
                nc, in_maps, core_ids=list(range(N_CORES)), tmpdir=tmpdir
            )
            break
        except Exception:
            # transient NRT_EXEC_UNIT_UNRECOVERABLE has been observed on a
            # first attempt; a clean retry recovers
            if attempt == 2:
                raise
            time.sleep(2.0)
    global last_results, last_tmpdir
    last_results = res
    last_tmpdir = tmpdir

    out = np.empty((B, S, E), dtype=np.float32)
    for c in range(N_CORES):
        b, h = divmod(c, 2)
        out[b, h * SH : (h + 1) * SH] = res.results[c]["out"].astype(np.float32)
    return out
